# revision 13
# baseline (speedup 1.0000x reference)
"""Trainium2 Bass kernel for nn_HGTD (CP tensor-completion loss).

Computation (see reference):
  N,D,T,biases sampled via mu + softplus(rho)*eps  (tiny, done on host)
  pred = einsum('nr,dr,tr->ndt', N, D, T) + Nb+Db+Tb         [200,200,500]
  loss1 = ||where(~mask, pred-flow, 0)||_F
  loss2 = l*(||N||+||D||+||T||)
  loss3 = l*||hs - N@N.T||_F
  loss4 = l*||ht - (N@N.T)[:,:,None]*sumD||_F
  return loss1+loss2+loss3+loss4, pred

Device strategy (8 cores, shard dim0 of flow/mask/ht/pred -> 25 rows each):
  - biases folded into the CP matmul by augmenting rank 30 -> 32:
      ND_aug = [N (x) D | ones | Nb+Db],  T_aug = [T | Tb | ones]
    so PE produces pred tiles [128, 500] directly in PSUM.
  - per tile: ACT copies pred PSUM->SBUF (for DMA out), DVE computes
    diff = pred - flow, zeroes masked entries via copy_predicated, ACT
    squares with accum_out giving per-row partial sums of squares.
  - loss4 via quadratic expansion sum((ht - r1)^2) = sum(ht^2)
      - 2*sum_d sumD[d]*Hd[d] + sum(NNt^2)*sum(sumD^2), with
    sum(ht^2) and Hd = ht^T @ NNt_rows accumulated on PE in PSUM.
  - scalar finishing (traces, sqrt, tiny norms) on host in float64.
"""

import numpy as np

DIM1, DIM2, DIM3, RANK = 200, 200, 500, 30
LAMDA = 0.001
NCORES = 8
ROWS_N = DIM1 // NCORES          # 25 n-rows per core
SLAB = ROWS_N * DIM2             # 5000 nd-rows per core
P = 128
NT = (SLAB + P - 1) // P         # 40 tiles (39x128 + 1x8)
KAUG = RANK + 2                  # 32
HD = DIM2                        # heter_time last axis (200)
HCH = ((0, 128), (128, HD))      # d-chunks for loss4 PE pass

_CACHE = {}


def _split_multi_waits(nc, mybir):
    """walrus in this container accepts only ONE sem-wait per instruction.
    For any instruction Tile scheduled with >1 waits, hoist the extras onto
    freshly inserted NoOps on the same engine immediately before it (each
    engine executes its stream in block order, so all waits still complete
    before the instruction runs)."""
    n_split = 0
    for f in nc.m.functions:
        for bb in f.blocks:
            insts = list(bb.instructions)
            out = []
            changed = False
            for inst in insts:
                si = inst.sync_info
                if si is not None and si.on_wait and len(si.on_wait) > 1:
                    waits = list(si.on_wait)
                    for k, w in enumerate(waits[:-1]):
                        out.append(mybir.InstNoOp(
                            name=f"{inst.name}-sw{k}",
                            engine=inst.engine, ins=[], outs=[],
                            sync_info=mybir.SyncInfo(on_wait=[w],
                                                     on_update=[]),
                        ))
                    inst.sync_info = mybir.SyncInfo(
                        on_wait=[waits[-1]],
                        on_update=list(si.on_update or []),
                    )
                    changed = True
                    n_split += 1
                out.append(inst)
            if changed:
                bb.instructions = out
    return n_split


def _build_program():
    from concourse import bass, tile
    import concourse.mybir as mybir

    f32 = mybir.dt.float32
    f32r = mybir.dt.float32r
    u8 = mybir.dt.uint8
    Sub = mybir.AluOpType.subtract
    Sq = mybir.ActivationFunctionType.Square

    nc = bass.Bass("TRN2", target_bir_lowering=False, debug=False,
                   num_devices=NCORES)

    flow = nc.dram_tensor("flow", [SLAB, DIM3], f32, kind="ExternalInput")
    maskt = nc.dram_tensor("maskt", [SLAB, DIM3], u8, kind="ExternalInput")
    hslab = nc.dram_tensor("hslab", [SLAB, HD], f32, kind="ExternalInput")
    ndaugT = nc.dram_tensor("ndaugT", [KAUG, SLAB], f32r, kind="ExternalInput")
    taugT = nc.dram_tensor("taugT", [KAUG, DIM3], f32r, kind="ExternalInput")
    nntc = nc.dram_tensor("nntc", [P, NT], f32, kind="ExternalInput")

    predo = nc.dram_tensor("predo", [SLAB, DIM3], f32, kind="ExternalOutput")
    rssq = nc.dram_tensor("rssq", [P, NT], f32, kind="ExternalOutput")
    hsqo = nc.dram_tensor("hsqo", [P, NT], f32, kind="ExternalOutput")
    hdo = nc.dram_tensor("hdo", [1, HD], f32, kind="ExternalOutput")

    # group G row-tiles of 128 into one DMA (1MB flow transfers) to cut
    # Sync-sequencer DIRECT2D issue count; tail rows handled per-tile.
    G = 4
    n_groups = SLAB // (P * G)            # 9 full groups (4608 rows)
    tail0 = n_groups * P * G              # 4608
    tail_tiles = []
    r = tail0
    while r < SLAB:
        rows = min(P, SLAB - r)
        tail_tiles.append((r, rows))
        r += rows                          # (4608,128)(4736,128)(4864,128)(4992,8)

    with tile.TileContext(nc) as tc:
        with (
            tc.tile_pool(name="const", bufs=1) as constp,
            tc.tile_pool(name="io", bufs=4) as iop,
            tc.tile_pool(name="ps", bufs=7, space="PSUM") as psp,
            tc.tile_pool(name="acc", bufs=1, space="PSUM") as accp,
        ):
            nd_sb = constp.tile([KAUG, SLAB], f32r)
            nc.sync.dma_start(nd_sb[:], ndaugT[:])
            ta_sb = constp.tile([KAUG, DIM3], f32r)
            nc.sync.dma_start(ta_sb[:], taugT[:])
            nn_sb = constp.tile([P, NT], f32)
            nc.sync.dma_start(nn_sb[:], nntc[:])
            zeros = constp.tile([P, G * DIM3], f32)
            nc.vector.memset(zeros[:], 0.0)
            rs_sb = constp.tile([P, NT], f32)
            nc.vector.memset(rs_sb[:], 0.0)

            hd_ps = accp.tile([1, HD], f32)
            hsq_sb = constp.tile([P, NT], f32)
            nc.vector.memset(hsq_sb[:], 0.0)

            def h_tile_matmuls(ht_ap, rows, j, first, last):
                # ht_ap: [rows, HD] slice; j = 128-row tile index for nnt col
                nc.tensor.matmul(hd_ps[:1, :], nn_sb[:rows, j:j + 1], ht_ap,
                                 start=first, stop=last,
                                 skip_group_check=True)

            def pred_sub(fl_ap, pd_ap, r0, rows, copy_on_dve):
                # matmul pred -> psum; copy psum->sbuf out; diff into fl
                pp = psp.tile([P, DIM3], f32, tag="predps")
                nc.tensor.matmul(pp[:rows], nd_sb[:, r0:r0 + rows], ta_sb[:],
                                 start=True, stop=True)
                if copy_on_dve:
                    nc.vector.tensor_copy(pd_ap, pp[:rows])
                else:
                    nc.scalar.copy(pd_ap, pp[:rows])
                nc.vector.tensor_tensor(fl_ap, pp[:rows], fl_ap, Sub)

            # ---- main loop over groups: pred + loss1 + loss4 fused ----
            for g in range(n_groups):
                r0 = P * G * g
                fl = iop.tile([P, G, DIM3], f32, tag="flow")
                nc.sync.dma_start(
                    fl[:], flow[r0:r0 + P * G, :].rearrange(
                        "(a p) t -> p a t", p=P))
                mk = iop.tile([P, G, DIM3], u8, tag="mask")
                nc.sync.dma_start(
                    mk[:], maskt[r0:r0 + P * G, :].rearrange(
                        "(a p) t -> p a t", p=P))
                ht = iop.tile([P, G, HD], f32, tag="h")
                nc.gpsimd.dma_start(
                    ht[:], hslab[r0:r0 + P * G, :].rearrange(
                        "(a p) t -> p a t", p=P))
                pd = iop.tile([P, G, DIM3], f32, tag="predsb")

                for a in range(G):
                    i = G * g + a
                    pred_sub(fl[:, a, :], pd[:, a, :], r0 + P * a, P,
                             copy_on_dve=(a == 0))
                    h_tile_matmuls(ht[:, a, :], P, i,
                                   first=(i == 0), last=False)
                nc.vector.copy_predicated(fl[:], mk[:], zeros[:])
                nc.scalar.activation(fl[:], fl[:], Sq,
                                     accum_out=rs_sb[:, g:g + 1])
                nc.scalar.activation(ht[:], ht[:], Sq,
                                     accum_out=hsq_sb[:, g:g + 1])

                nc.scalar.dma_start(
                    predo[r0:r0 + P * G, :].rearrange("(a p) t -> p a t", p=P),
                    pd[:])

            # ---- ragged tail, per 128-row tile ----
            for k, (r0, rows) in enumerate(tail_tiles):
                i = n_groups * G + k
                fl = iop.tile([P, DIM3], f32, tag="tflow")
                nc.sync.dma_start(fl[:rows], flow[r0:r0 + rows, :])
                mk = iop.tile([P, DIM3], u8, tag="tmask")
                nc.sync.dma_start(mk[:rows], maskt[r0:r0 + rows, :])
                ht = iop.tile([P, HD], f32, tag="th")
                nc.gpsimd.dma_start(ht[:rows], hslab[r0:r0 + rows, :])
                pd = iop.tile([P, DIM3], f32, tag="tpred")
                pred_sub(fl[:rows], pd[:rows], r0, rows, copy_on_dve=False)
                nc.vector.copy_predicated(fl[:rows], mk[:rows],
                                          zeros[:rows, :DIM3])
                nc.scalar.activation(fl[:rows], fl[:rows], Sq,
                                     accum_out=rs_sb[:rows, i:i + 1])
                h_tile_matmuls(ht[:rows], rows, i,
                               first=False, last=(k == len(tail_tiles) - 1))
                nc.scalar.activation(ht[:rows], ht[:rows], Sq,
                                     accum_out=hsq_sb[:rows, i:i + 1])
                nc.scalar.dma_start(predo[r0:r0 + rows, :], pd[:rows])

            # ---- epilogue: small outputs ----
            nc.sync.dma_start(hsqo[:, :], hsq_sb[:])
            hd_sb = constp.tile([1, HD], f32)
            nc.scalar.copy(hd_sb[:], hd_ps[:])
            nc.sync.dma_start(hdo[:, :], hd_sb[:])
            nc.sync.dma_start(rssq[:, :], rs_sb[:])

    _split_multi_waits(nc, mybir)
    return nc


def _get_program():
    if "nc" not in _CACHE:
        _CACHE["nc"] = _build_program()
    return _CACHE["nc"]


def _sample(mu, rho, eps):
    mu = np.asarray(mu, np.float32)
    rho = np.asarray(rho, np.float32)
    eps = np.asarray(eps, np.float32)
    return mu + np.log1p(np.exp(rho)) * eps


def _prep_inputs(flow_missing, flow_missing_mask, heter_time_unmasked,
                 N, D, T, N_bias, D_bias, T_bias):
    """Build the per-core input maps."""
    flow = np.ascontiguousarray(np.asarray(flow_missing, np.float32))
    mask = np.asarray(flow_missing_mask)
    if mask.dtype != np.uint8:
        mask = mask.astype(np.uint8)
    ht = np.ascontiguousarray(np.asarray(heter_time_unmasked, np.float32))

    # augmented factors: pred = ND_aug @ T_aug^T
    ndkr = (N[:, None, :] * D[None, :, :]).reshape(DIM1 * DIM2, RANK)
    nbdb = (N_bias[:, None] + D_bias[None, :]).reshape(DIM1 * DIM2)
    nd_aug = np.empty((DIM1 * DIM2, KAUG), np.float32)
    nd_aug[:, :RANK] = ndkr
    nd_aug[:, RANK] = 1.0
    nd_aug[:, RANK + 1] = nbdb
    ndaugT = np.ascontiguousarray(nd_aug.T)          # [32, 40000]

    t_aug = np.empty((DIM3, KAUG), np.float32)
    t_aug[:, :RANK] = T
    t_aug[:, RANK] = T_bias
    t_aug[:, RANK + 1] = 1.0
    taugT = np.ascontiguousarray(t_aug.T)            # [32, 500]

    nnt = (N @ N.T).astype(np.float32)               # [200, 200]

    in_maps = []
    for c in range(NCORES):
        n0 = ROWS_N * c
        nn_flat = np.ascontiguousarray(nnt[n0:n0 + ROWS_N]).reshape(SLAB)
        nn_cols = np.zeros((P, NT), np.float32)
        nn_cols.T.reshape(-1)[:SLAB] = nn_flat       # col j = rows 128j..128j+128
        in_maps.append({
            "flow": flow[n0:n0 + ROWS_N].reshape(SLAB, DIM3),
            "maskt": np.ascontiguousarray(
                mask[n0:n0 + ROWS_N].reshape(SLAB, DIM3)),
            "hslab": ht[n0:n0 + ROWS_N].reshape(SLAB, HD),
            "ndaugT": np.ascontiguousarray(
                ndaugT[:, SLAB * c:SLAB * (c + 1)]),
            "taugT": taugT,
            "nntc": nn_cols,
        })
    return in_maps, nnt


def kernel(flow_missing, flow_missing_mask, heter_spatial_unmasked,
           heter_time_unmasked,
           N_mu, N_rho, eps_N, D_mu, D_rho, eps_D, T_mu, T_rho, eps_T,
           Nb_mu, Nb_rho, eps_Nb, Db_mu, Db_rho, eps_Db,
           Tb_mu, Tb_rho, eps_Tb,
           _trace=False, _trace_kwargs=None):
    from concourse.bass_utils import run_bass_kernel_spmd

    N = _sample(N_mu, N_rho, eps_N)
    D = _sample(D_mu, D_rho, eps_D)
    T = _sample(T_mu, T_rho, eps_T)
    N_bias = _sample(Nb_mu, Nb_rho, eps_Nb)
    D_bias = _sample(Db_mu, Db_rho, eps_Db)
    T_bias = _sample(Tb_mu, Tb_rho, eps_Tb)

    in_maps, nnt = _prep_inputs(flow_missing, flow_missing_mask,
                                heter_time_unmasked,
                                N, D, T, N_bias, D_bias, T_bias)

    nc = _get_program()
    res = run_bass_kernel_spmd(nc, in_maps, list(range(NCORES)),
                               trace=_trace, **(_trace_kwargs or {}))
    _CACHE["last_res"] = res
    results = res.results

    pred = np.concatenate(
        [results[c]["predo"].reshape(ROWS_N, DIM2, DIM3)
         for c in range(NCORES)], axis=0)

    # loss1
    ssq1 = np.float64(0.0)
    for c in range(NCORES):
        ssq1 += results[c]["rssq"].astype(np.float64).sum()
    loss1 = np.sqrt(ssq1)

    # loss2 (host, tiny)
    def fro(x):
        return np.sqrt(np.sum(np.asarray(x, np.float64) ** 2))
    loss2 = LAMDA * (fro(N) + fro(D) + fro(T))

    # loss3 (host, tiny)
    hs = np.asarray(heter_spatial_unmasked, np.float64)
    loss3 = LAMDA * fro(hs - nnt.astype(np.float64))

    # loss4 from device partials
    sumD = D.astype(np.float64).sum(axis=1)          # [200]
    ssq_ht = np.float64(0.0)
    hd_tot = np.zeros(HD, np.float64)
    for c in range(NCORES):
        ssq_ht += results[c]["hsqo"].astype(np.float64).sum()
        hd_tot += results[c]["hdo"].astype(np.float64).reshape(HD)
    ssq4 = (ssq_ht - 2.0 * np.dot(sumD, hd_tot)
            + np.sum(nnt.astype(np.float64) ** 2) * np.sum(sumD ** 2))
    loss4 = LAMDA * np.sqrt(ssq4)

    loss = np.float32(loss1 + loss2 + loss3 + loss4)
    return loss, pred


if __name__ == "__main__":
    # quick self-exercise with random data
    rng = np.random.default_rng(0)
    ins = {
        "flow_missing": rng.standard_normal((DIM1, DIM2, DIM3), np.float32),
        "flow_missing_mask": rng.integers(0, 2, (DIM1, DIM2, DIM3)).astype(bool),
        "heter_spatial_unmasked": rng.standard_normal((DIM1, DIM1), np.float32),
        "heter_time_unmasked": rng.standard_normal((DIM1, DIM1, DIM2), np.float32),
    }
    for nm, shp in (("N", (DIM1, RANK)), ("D", (DIM2, RANK)), ("T", (DIM3, RANK))):
        ins[f"{nm}_mu"] = (rng.standard_normal(shp) * 0.1).astype(np.float32)
        ins[f"{nm}_rho"] = (rng.standard_normal(shp) * 0.1 - 7).astype(np.float32)
        ins[f"eps_{nm}"] = rng.standard_normal(shp).astype(np.float32)
    for nm, shp in (("Nb", (DIM1,)), ("Db", (DIM2,)), ("Tb", (DIM3,))):
        ins[f"{nm}_mu"] = (rng.standard_normal(shp) * 0.1).astype(np.float32)
        ins[f"{nm}_rho"] = (rng.standard_normal(shp) * 0.1 - 7).astype(np.float32)
        ins[f"eps_{nm}"] = rng.standard_normal(shp).astype(np.float32)
    loss, pred = kernel(**ins)
    print("loss:", loss, "pred:", pred.shape, pred.dtype)


# revision 14
# speedup vs baseline: 1.1421x; 1.1421x over previous
"""Trainium2 Bass kernel for nn_HGTD (CP tensor-completion loss).

Computation (see reference):
  N,D,T,biases sampled via mu + softplus(rho)*eps  (tiny, done on host)
  pred = einsum('nr,dr,tr->ndt', N, D, T) + Nb+Db+Tb         [200,200,500]
  loss1 = ||where(~mask, pred-flow, 0)||_F
  loss2 = l*(||N||+||D||+||T||)
  loss3 = l*||hs - N@N.T||_F
  loss4 = l*||ht - (N@N.T)[:,:,None]*sumD||_F
  return loss1+loss2+loss3+loss4, pred

Device strategy (8 cores, shard dim0 of flow/mask/ht/pred -> 25 rows each):
  - biases folded into the CP matmul by augmenting rank 30 -> 32:
      ND_aug = [N (x) D | ones | Nb+Db],  T_aug = [T | Tb | ones]
    so PE produces pred tiles [128, 500] directly in PSUM.
  - per tile: ACT copies pred PSUM->SBUF (for DMA out), DVE computes
    diff = pred - flow, zeroes masked entries via copy_predicated, ACT
    squares with accum_out giving per-row partial sums of squares.
  - loss4 via quadratic expansion sum((ht - r1)^2) = sum(ht^2)
      - 2*sum_d sumD[d]*Hd[d] + sum(NNt^2)*sum(sumD^2), with
    sum(ht^2) and Hd = ht^T @ NNt_rows accumulated on PE in PSUM.
  - scalar finishing (traces, sqrt, tiny norms) on host in float64.
"""

import numpy as np

DIM1, DIM2, DIM3, RANK = 200, 200, 500, 30
LAMDA = 0.001
NCORES = 8
ROWS_N = DIM1 // NCORES          # 25 n-rows per core
SLAB = ROWS_N * DIM2             # 5000 nd-rows per core
P = 128
NT = (SLAB + P - 1) // P         # 40 tiles (39x128 + 1x8)
KAUG = RANK + 2                  # 32
HD = DIM2                        # heter_time last axis (200)
HCH = ((0, 128), (128, HD))      # d-chunks for loss4 PE pass

_CACHE = {}


def _split_multi_waits(nc, mybir):
    """walrus in this container accepts only ONE sem-wait per instruction.
    For any instruction Tile scheduled with >1 waits, hoist the extras onto
    freshly inserted NoOps on the same engine immediately before it (each
    engine executes its stream in block order, so all waits still complete
    before the instruction runs)."""
    n_split = 0
    for f in nc.m.functions:
        for bb in f.blocks:
            insts = list(bb.instructions)
            out = []
            changed = False
            for inst in insts:
                si = inst.sync_info
                if si is not None and si.on_wait and len(si.on_wait) > 1:
                    waits = list(si.on_wait)
                    for k, w in enumerate(waits[:-1]):
                        out.append(mybir.InstNoOp(
                            name=f"{inst.name}-sw{k}",
                            engine=inst.engine, ins=[], outs=[],
                            sync_info=mybir.SyncInfo(on_wait=[w],
                                                     on_update=[]),
                        ))
                    inst.sync_info = mybir.SyncInfo(
                        on_wait=[waits[-1]],
                        on_update=list(si.on_update or []),
                    )
                    changed = True
                    n_split += 1
                out.append(inst)
            if changed:
                bb.instructions = out
    return n_split


def _build_program():
    from concourse import bass, tile
    import concourse.mybir as mybir

    f32 = mybir.dt.float32
    f32r = mybir.dt.float32r
    u8 = mybir.dt.uint8
    Sub = mybir.AluOpType.subtract
    Sq = mybir.ActivationFunctionType.Square

    nc = bass.Bass("TRN2", target_bir_lowering=False, debug=False,
                   num_devices=NCORES)

    flow = nc.dram_tensor("flow", [SLAB, DIM3], f32, kind="ExternalInput")
    maskt = nc.dram_tensor("maskt", [SLAB, DIM3], u8, kind="ExternalInput")
    hslab = nc.dram_tensor("hslab", [SLAB, HD], f32, kind="ExternalInput")
    ndaugT = nc.dram_tensor("ndaugT", [KAUG, SLAB], f32r, kind="ExternalInput")
    taugT = nc.dram_tensor("taugT", [KAUG, DIM3], f32r, kind="ExternalInput")
    nntc = nc.dram_tensor("nntc", [P, NT], f32, kind="ExternalInput")

    predo = nc.dram_tensor("predo", [SLAB, DIM3], f32, kind="ExternalOutput")
    rssq = nc.dram_tensor("rssq", [P, NT], f32, kind="ExternalOutput")
    hsqo = nc.dram_tensor("hsqo", [P, NT], f32, kind="ExternalOutput")
    hdo = nc.dram_tensor("hdo", [1, HD], f32, kind="ExternalOutput")

    # group G row-tiles of 128 into one DMA (1MB flow transfers) to cut
    # Sync-sequencer DIRECT2D issue count; tail rows handled per-tile.
    G = 4
    n_groups = SLAB // (P * G)            # 9 full groups (4608 rows)
    tail0 = n_groups * P * G              # 4608
    tail_tiles = []
    r = tail0
    while r < SLAB:
        rows = min(P, SLAB - r)
        tail_tiles.append((r, rows))
        r += rows                          # (4608,128)(4736,128)(4864,128)(4992,8)

    with tile.TileContext(nc) as tc:
        with (
            tc.tile_pool(name="const", bufs=1) as constp,
            tc.tile_pool(name="io", bufs=4) as iop,
            tc.tile_pool(name="ps", bufs=4, space="PSUM") as psp,
            tc.tile_pool(name="acc", bufs=1, space="PSUM") as accp,
        ):
            nd_sb = constp.tile([KAUG, SLAB], f32r)
            nc.sync.dma_start(nd_sb[:], ndaugT[:])
            ta_sb = constp.tile([KAUG, DIM3], f32r)
            nc.sync.dma_start(ta_sb[:], taugT[:])
            nn_sb = constp.tile([P, NT], f32)
            nc.sync.dma_start(nn_sb[:], nntc[:])
            zeros = constp.tile([P, G * DIM3], f32)
            nc.vector.memset(zeros[:], 0.0)
            rs_sb = constp.tile([P, NT], f32)
            nc.vector.memset(rs_sb[:], 0.0)

            hd_ps = accp.tile([1, HD], f32)
            hsq_sb = constp.tile([P, NT], f32)
            nc.vector.memset(hsq_sb[:], 0.0)

            def h_tile_matmuls(ht_ap, rows, j, first, last):
                # ht_ap: [rows, HD] slice; j = 128-row tile index for nnt col
                nc.tensor.matmul(hd_ps[:1, :], nn_sb[:rows, j:j + 1], ht_ap,
                                 start=first, stop=last,
                                 skip_group_check=True)

            def pred_sub(fl_ap, pd_ap, r0, rows, copy_on_dve):
                # matmul pred -> psum; copy psum->sbuf out; diff into fl
                pp = psp.tile([P, DIM3], f32, tag="predps")
                nc.tensor.matmul(pp[:rows], nd_sb[:, r0:r0 + rows], ta_sb[:],
                                 start=True, stop=True)
                if copy_on_dve:
                    nc.vector.tensor_copy(pd_ap, pp[:rows])
                else:
                    nc.scalar.copy(pd_ap, pp[:rows])
                nc.vector.tensor_tensor(fl_ap, pp[:rows], fl_ap, Sub)

            # ---- main loop over groups: pred + loss1 + loss4 fused ----
            # stage-2 (masked square / H square) delayed one group so the
            # in-order DVE/ACT streams don't stall at intra-group barriers
            pending = None
            for g in range(n_groups):
                r0 = P * G * g
                fl = iop.tile([P, G, DIM3], f32, tag="flow")
                nc.sync.dma_start(
                    fl[:], flow[r0:r0 + P * G, :].rearrange(
                        "(a p) t -> p a t", p=P))
                mk = iop.tile([P, G, DIM3], u8, tag="mask")
                nc.sync.dma_start(
                    mk[:], maskt[r0:r0 + P * G, :].rearrange(
                        "(a p) t -> p a t", p=P))
                ht = iop.tile([P, G, HD], f32, tag="h")
                nc.gpsimd.dma_start(
                    ht[:], hslab[r0:r0 + P * G, :].rearrange(
                        "(a p) t -> p a t", p=P))
                pd = iop.tile([P, G, DIM3], f32, tag="predsb")

                for a in range(G):
                    i = G * g + a
                    pred_sub(fl[:, a, :], pd[:, a, :], r0 + P * a, P,
                             copy_on_dve=(a == 0))
                    h_tile_matmuls(ht[:, a, :], P, i,
                                   first=(i == 0), last=False)
                nc.scalar.dma_start(
                    predo[r0:r0 + P * G, :].rearrange("(a p) t -> p a t", p=P),
                    pd[:])

                if pending is not None:
                    pfl, pmk, pht, pg = pending
                    nc.vector.copy_predicated(pfl[:], pmk[:], zeros[:])
                    nc.scalar.activation(pfl[:], pfl[:], Sq,
                                         accum_out=rs_sb[:, pg:pg + 1])
                    nc.scalar.activation(pht[:], pht[:], Sq,
                                         accum_out=hsq_sb[:, pg:pg + 1])
                pending = (fl, mk, ht, g)

            pfl, pmk, pht, pg = pending
            nc.vector.copy_predicated(pfl[:], pmk[:], zeros[:])
            nc.scalar.activation(pfl[:], pfl[:], Sq,
                                 accum_out=rs_sb[:, pg:pg + 1])
            nc.scalar.activation(pht[:], pht[:], Sq,
                                 accum_out=hsq_sb[:, pg:pg + 1])

            # ---- ragged tail, per 128-row tile ----
            for k, (r0, rows) in enumerate(tail_tiles):
                i = n_groups * G + k
                fl = iop.tile([P, DIM3], f32, tag="tflow")
                nc.sync.dma_start(fl[:rows], flow[r0:r0 + rows, :])
                mk = iop.tile([P, DIM3], u8, tag="tmask")
                nc.sync.dma_start(mk[:rows], maskt[r0:r0 + rows, :])
                ht = iop.tile([P, HD], f32, tag="th")
                nc.gpsimd.dma_start(ht[:rows], hslab[r0:r0 + rows, :])
                pd = iop.tile([P, DIM3], f32, tag="tpred")
                pred_sub(fl[:rows], pd[:rows], r0, rows, copy_on_dve=False)
                nc.vector.copy_predicated(fl[:rows], mk[:rows],
                                          zeros[:rows, :DIM3])
                nc.scalar.activation(fl[:rows], fl[:rows], Sq,
                                     accum_out=rs_sb[:rows, i:i + 1])
                h_tile_matmuls(ht[:rows], rows, i,
                               first=False, last=(k == len(tail_tiles) - 1))
                nc.scalar.activation(ht[:rows], ht[:rows], Sq,
                                     accum_out=hsq_sb[:rows, i:i + 1])
                nc.scalar.dma_start(predo[r0:r0 + rows, :], pd[:rows])

            # ---- epilogue: small outputs ----
            nc.sync.dma_start(hsqo[:, :], hsq_sb[:])
            hd_sb = constp.tile([1, HD], f32)
            nc.scalar.copy(hd_sb[:], hd_ps[:])
            nc.sync.dma_start(hdo[:, :], hd_sb[:])
            nc.sync.dma_start(rssq[:, :], rs_sb[:])

    _split_multi_waits(nc, mybir)
    return nc


def _get_program():
    if "nc" not in _CACHE:
        _CACHE["nc"] = _build_program()
    return _CACHE["nc"]


def _sample(mu, rho, eps):
    mu = np.asarray(mu, np.float32)
    rho = np.asarray(rho, np.float32)
    eps = np.asarray(eps, np.float32)
    return mu + np.log1p(np.exp(rho)) * eps


def _prep_inputs(flow_missing, flow_missing_mask, heter_time_unmasked,
                 N, D, T, N_bias, D_bias, T_bias):
    """Build the per-core input maps."""
    flow = np.ascontiguousarray(np.asarray(flow_missing, np.float32))
    mask = np.asarray(flow_missing_mask)
    if mask.dtype != np.uint8:
        mask = mask.astype(np.uint8)
    ht = np.ascontiguousarray(np.asarray(heter_time_unmasked, np.float32))

    # augmented factors: pred = ND_aug @ T_aug^T
    ndkr = (N[:, None, :] * D[None, :, :]).reshape(DIM1 * DIM2, RANK)
    nbdb = (N_bias[:, None] + D_bias[None, :]).reshape(DIM1 * DIM2)
    nd_aug = np.empty((DIM1 * DIM2, KAUG), np.float32)
    nd_aug[:, :RANK] = ndkr
    nd_aug[:, RANK] = 1.0
    nd_aug[:, RANK + 1] = nbdb
    ndaugT = np.ascontiguousarray(nd_aug.T)          # [32, 40000]

    t_aug = np.empty((DIM3, KAUG), np.float32)
    t_aug[:, :RANK] = T
    t_aug[:, RANK] = T_bias
    t_aug[:, RANK + 1] = 1.0
    taugT = np.ascontiguousarray(t_aug.T)            # [32, 500]

    nnt = (N @ N.T).astype(np.float32)               # [200, 200]

    in_maps = []
    for c in range(NCORES):
        n0 = ROWS_N * c
        nn_flat = np.ascontiguousarray(nnt[n0:n0 + ROWS_N]).reshape(SLAB)
        nn_cols = np.zeros((P, NT), np.float32)
        nn_cols.T.reshape(-1)[:SLAB] = nn_flat       # col j = rows 128j..128j+128
        in_maps.append({
            "flow": flow[n0:n0 + ROWS_N].reshape(SLAB, DIM3),
            "maskt": np.ascontiguousarray(
                mask[n0:n0 + ROWS_N].reshape(SLAB, DIM3)),
            "hslab": ht[n0:n0 + ROWS_N].reshape(SLAB, HD),
            "ndaugT": np.ascontiguousarray(
                ndaugT[:, SLAB * c:SLAB * (c + 1)]),
            "taugT": taugT,
            "nntc": nn_cols,
        })
    return in_maps, nnt


def kernel(flow_missing, flow_missing_mask, heter_spatial_unmasked,
           heter_time_unmasked,
           N_mu, N_rho, eps_N, D_mu, D_rho, eps_D, T_mu, T_rho, eps_T,
           Nb_mu, Nb_rho, eps_Nb, Db_mu, Db_rho, eps_Db,
           Tb_mu, Tb_rho, eps_Tb,
           _trace=False, _trace_kwargs=None):
    from concourse.bass_utils import run_bass_kernel_spmd

    N = _sample(N_mu, N_rho, eps_N)
    D = _sample(D_mu, D_rho, eps_D)
    T = _sample(T_mu, T_rho, eps_T)
    N_bias = _sample(Nb_mu, Nb_rho, eps_Nb)
    D_bias = _sample(Db_mu, Db_rho, eps_Db)
    T_bias = _sample(Tb_mu, Tb_rho, eps_Tb)

    in_maps, nnt = _prep_inputs(flow_missing, flow_missing_mask,
                                heter_time_unmasked,
                                N, D, T, N_bias, D_bias, T_bias)

    nc = _get_program()
    res = run_bass_kernel_spmd(nc, in_maps, list(range(NCORES)),
                               trace=_trace, **(_trace_kwargs or {}))
    _CACHE["last_res"] = res
    results = res.results

    pred = np.concatenate(
        [results[c]["predo"].reshape(ROWS_N, DIM2, DIM3)
         for c in range(NCORES)], axis=0)

    # loss1
    ssq1 = np.float64(0.0)
    for c in range(NCORES):
        ssq1 += results[c]["rssq"].astype(np.float64).sum()
    loss1 = np.sqrt(ssq1)

    # loss2 (host, tiny)
    def fro(x):
        return np.sqrt(np.sum(np.asarray(x, np.float64) ** 2))
    loss2 = LAMDA * (fro(N) + fro(D) + fro(T))

    # loss3 (host, tiny)
    hs = np.asarray(heter_spatial_unmasked, np.float64)
    loss3 = LAMDA * fro(hs - nnt.astype(np.float64))

    # loss4 from device partials
    sumD = D.astype(np.float64).sum(axis=1)          # [200]
    ssq_ht = np.float64(0.0)
    hd_tot = np.zeros(HD, np.float64)
    for c in range(NCORES):
        ssq_ht += results[c]["hsqo"].astype(np.float64).sum()
        hd_tot += results[c]["hdo"].astype(np.float64).reshape(HD)
    ssq4 = (ssq_ht - 2.0 * np.dot(sumD, hd_tot)
            + np.sum(nnt.astype(np.float64) ** 2) * np.sum(sumD ** 2))
    loss4 = LAMDA * np.sqrt(ssq4)

    loss = np.float32(loss1 + loss2 + loss3 + loss4)
    return loss, pred


if __name__ == "__main__":
    # quick self-exercise with random data
    rng = np.random.default_rng(0)
    ins = {
        "flow_missing": rng.standard_normal((DIM1, DIM2, DIM3), np.float32),
        "flow_missing_mask": rng.integers(0, 2, (DIM1, DIM2, DIM3)).astype(bool),
        "heter_spatial_unmasked": rng.standard_normal((DIM1, DIM1), np.float32),
        "heter_time_unmasked": rng.standard_normal((DIM1, DIM1, DIM2), np.float32),
    }
    for nm, shp in (("N", (DIM1, RANK)), ("D", (DIM2, RANK)), ("T", (DIM3, RANK))):
        ins[f"{nm}_mu"] = (rng.standard_normal(shp) * 0.1).astype(np.float32)
        ins[f"{nm}_rho"] = (rng.standard_normal(shp) * 0.1 - 7).astype(np.float32)
        ins[f"eps_{nm}"] = rng.standard_normal(shp).astype(np.float32)
    for nm, shp in (("Nb", (DIM1,)), ("Db", (DIM2,)), ("Tb", (DIM3,))):
        ins[f"{nm}_mu"] = (rng.standard_normal(shp) * 0.1).astype(np.float32)
        ins[f"{nm}_rho"] = (rng.standard_normal(shp) * 0.1 - 7).astype(np.float32)
        ins[f"eps_{nm}"] = rng.standard_normal(shp).astype(np.float32)
    loss, pred = kernel(**ins)
    print("loss:", loss, "pred:", pred.shape, pred.dtype)


# revision 15
# speedup vs baseline: 1.2100x; 1.0595x over previous
"""Trainium2 Bass kernel for nn_HGTD (CP tensor-completion loss).

Computation (see reference):
  N,D,T,biases sampled via mu + softplus(rho)*eps  (tiny, done on host)
  pred = einsum('nr,dr,tr->ndt', N, D, T) + Nb+Db+Tb         [200,200,500]
  loss1 = ||where(~mask, pred-flow, 0)||_F
  loss2 = l*(||N||+||D||+||T||)
  loss3 = l*||hs - N@N.T||_F
  loss4 = l*||ht - (N@N.T)[:,:,None]*sumD||_F
  return loss1+loss2+loss3+loss4, pred

Device strategy (8 cores, shard dim0 of flow/mask/ht/pred -> 25 rows each):
  - biases folded into the CP matmul by augmenting rank 30 -> 32:
      ND_aug = [N (x) D | ones | Nb+Db],  T_aug = [T | Tb | ones]
    so PE produces pred tiles [128, 500] directly in PSUM.
  - per tile: ACT copies pred PSUM->SBUF (for DMA out), DVE computes
    diff = pred - flow, zeroes masked entries via copy_predicated, ACT
    squares with accum_out giving per-row partial sums of squares.
  - loss4 via quadratic expansion sum((ht - r1)^2) = sum(ht^2)
      - 2*sum_d sumD[d]*Hd[d] + sum(NNt^2)*sum(sumD^2), with
    sum(ht^2) and Hd = ht^T @ NNt_rows accumulated on PE in PSUM.
  - scalar finishing (traces, sqrt, tiny norms) on host in float64.
"""

import numpy as np

DIM1, DIM2, DIM3, RANK = 200, 200, 500, 30
LAMDA = 0.001
NCORES = 8
ROWS_N = DIM1 // NCORES          # 25 n-rows per core
SLAB = ROWS_N * DIM2             # 5000 nd-rows per core
P = 128
NT = (SLAB + P - 1) // P         # 40 tiles (39x128 + 1x8)
KAUG = RANK + 2                  # 32
HD = DIM2                        # heter_time last axis (200)
HCH = ((0, 128), (128, HD))      # d-chunks for loss4 PE pass

_CACHE = {}


def _split_multi_waits(nc, mybir):
    """walrus in this container accepts only ONE sem-wait per instruction.
    For any instruction Tile scheduled with >1 waits, hoist the extras onto
    freshly inserted NoOps on the same engine immediately before it (each
    engine executes its stream in block order, so all waits still complete
    before the instruction runs)."""
    n_split = 0
    for f in nc.m.functions:
        for bb in f.blocks:
            insts = list(bb.instructions)
            out = []
            changed = False
            for inst in insts:
                si = inst.sync_info
                if si is not None and si.on_wait and len(si.on_wait) > 1:
                    waits = list(si.on_wait)
                    for k, w in enumerate(waits[:-1]):
                        out.append(mybir.InstNoOp(
                            name=f"{inst.name}-sw{k}",
                            engine=inst.engine, ins=[], outs=[],
                            sync_info=mybir.SyncInfo(on_wait=[w],
                                                     on_update=[]),
                        ))
                    inst.sync_info = mybir.SyncInfo(
                        on_wait=[waits[-1]],
                        on_update=list(si.on_update or []),
                    )
                    changed = True
                    n_split += 1
                out.append(inst)
            if changed:
                bb.instructions = out
    return n_split


def _build_program():
    from concourse import bass, tile
    import concourse.mybir as mybir

    f32 = mybir.dt.float32
    f32r = mybir.dt.float32r
    u8 = mybir.dt.uint8
    Sub = mybir.AluOpType.subtract
    Sq = mybir.ActivationFunctionType.Square

    nc = bass.Bass("TRN2", target_bir_lowering=False, debug=False,
                   num_devices=NCORES)

    flow = nc.dram_tensor("flow", [SLAB, DIM3], f32, kind="ExternalInput")
    maskt = nc.dram_tensor("maskt", [SLAB, DIM3], u8, kind="ExternalInput")
    hslab = nc.dram_tensor("hslab", [SLAB, HD], f32, kind="ExternalInput")
    ndaugT = nc.dram_tensor("ndaugT", [KAUG, SLAB], f32r, kind="ExternalInput")
    taugT = nc.dram_tensor("taugT", [KAUG, DIM3], f32r, kind="ExternalInput")
    nntc = nc.dram_tensor("nntc", [P, NT], f32, kind="ExternalInput")

    predo = nc.dram_tensor("predo", [SLAB, DIM3], f32, kind="ExternalOutput")
    rssq = nc.dram_tensor("rssq", [P, NT], f32, kind="ExternalOutput")
    hsqo = nc.dram_tensor("hsqo", [P, NT], f32, kind="ExternalOutput")
    hdo = nc.dram_tensor("hdo", [1, HD], f32, kind="ExternalOutput")

    # group G row-tiles of 128 into one DMA (1MB flow transfers) to cut
    # Sync-sequencer DIRECT2D issue count; tail rows handled per-tile.
    G = 4
    n_groups = SLAB // (P * G)            # 9 full groups (4608 rows)
    tail0 = n_groups * P * G              # 4608
    tail_tiles = []
    r = tail0
    while r < SLAB:
        rows = min(P, SLAB - r)
        tail_tiles.append((r, rows))
        r += rows                          # (4608,128)(4736,128)(4864,128)(4992,8)

    with tile.TileContext(nc) as tc:
        with (
            tc.tile_pool(name="const", bufs=1) as constp,
            tc.tile_pool(name="io", bufs=4) as iop,
            tc.tile_pool(name="ps", bufs=4, space="PSUM") as psp,
            tc.tile_pool(name="acc", bufs=1, space="PSUM") as accp,
        ):
            nd_sb = constp.tile([KAUG, SLAB], f32r)
            nc.sync.dma_start(nd_sb[:], ndaugT[:])
            ta_sb = constp.tile([KAUG, DIM3], f32r)
            nc.sync.dma_start(ta_sb[:], taugT[:])
            nn_sb = constp.tile([P, NT], f32)
            nc.sync.dma_start(nn_sb[:], nntc[:])
            zeros = constp.tile([P, G * DIM3], f32)
            nc.vector.memset(zeros[:], 0.0)
            rs_sb = constp.tile([P, NT], f32)
            nc.vector.memset(rs_sb[:], 0.0)

            hd_ps = accp.tile([1, HD], f32)
            hsq_sb = constp.tile([P, NT], f32)
            nc.vector.memset(hsq_sb[:], 0.0)

            def h_tile_matmuls(ht_ap, rows, j, first, last):
                # ht_ap: [rows, HD] slice; j = 128-row tile index for nnt col
                nc.tensor.matmul(hd_ps[:1, :], nn_sb[:rows, j:j + 1], ht_ap,
                                 start=first, stop=last,
                                 skip_group_check=True)

            def pred_sub(fl_ap, pd_ap, r0, rows, copy_on_dve):
                # matmul pred -> psum; copy psum->sbuf out; diff into fl
                pp = psp.tile([P, DIM3], f32, tag="predps")
                nc.tensor.matmul(pp[:rows], nd_sb[:, r0:r0 + rows], ta_sb[:],
                                 start=True, stop=True)
                if copy_on_dve:
                    nc.vector.tensor_copy(pd_ap, pp[:rows])
                else:
                    nc.scalar.copy(pd_ap, pp[:rows])
                nc.vector.tensor_tensor(fl_ap, pp[:rows], fl_ap, Sub)

            # ---- ragged tail first (hides under the startup ramp):
            # 3 full 128-row tiles as one grouped DMA + one 8-row tile ----
            t0r = n_groups * P * G                      # 4608
            fl3 = iop.tile([P, 3, DIM3], f32, tag="tflow")
            nc.sync.dma_start(fl3[:], flow[t0r:t0r + 3 * P, :].rearrange(
                "(a p) t -> p a t", p=P))
            mk3 = iop.tile([P, 3, DIM3], u8, tag="tmask")
            nc.sync.dma_start(mk3[:], maskt[t0r:t0r + 3 * P, :].rearrange(
                "(a p) t -> p a t", p=P))
            ht3 = iop.tile([P, 3, HD], f32, tag="th")
            nc.gpsimd.dma_start(ht3[:], hslab[t0r:t0r + 3 * P, :].rearrange(
                "(a p) t -> p a t", p=P))
            pd3 = iop.tile([P, 3, DIM3], f32, tag="tpred")
            for a in range(3):
                i = n_groups * G + a
                pred_sub(fl3[:, a, :], pd3[:, a, :], t0r + P * a, P,
                         copy_on_dve=(a == 0))
                h_tile_matmuls(ht3[:, a, :], P, i, first=(a == 0), last=False)
            nc.scalar.dma_start(
                predo[t0r:t0r + 3 * P, :].rearrange("(a p) t -> p a t", p=P),
                pd3[:])
            nc.vector.copy_predicated(fl3[:], mk3[:], zeros[:, :3 * DIM3])
            nc.scalar.activation(fl3[:], fl3[:], Sq,
                                 accum_out=rs_sb[:, NT - 2:NT - 1])
            nc.scalar.activation(ht3[:], ht3[:], Sq,
                                 accum_out=hsq_sb[:, NT - 2:NT - 1])

            r8, rows8 = tail_tiles[-1]                  # (4992, 8)
            fl8 = iop.tile([P, DIM3], f32, tag="t8flow")
            nc.sync.dma_start(fl8[:rows8], flow[r8:r8 + rows8, :])
            mk8 = iop.tile([P, DIM3], u8, tag="t8mask")
            nc.sync.dma_start(mk8[:rows8], maskt[r8:r8 + rows8, :])
            ht8 = iop.tile([P, HD], f32, tag="t8h")
            nc.gpsimd.dma_start(ht8[:rows8], hslab[r8:r8 + rows8, :])
            pd8 = iop.tile([P, DIM3], f32, tag="t8pred")
            pred_sub(fl8[:rows8], pd8[:rows8], r8, rows8, copy_on_dve=False)
            nc.vector.copy_predicated(fl8[:rows8], mk8[:rows8],
                                      zeros[:rows8, :DIM3])
            nc.scalar.activation(fl8[:rows8], fl8[:rows8], Sq,
                                 accum_out=rs_sb[:rows8, NT - 1:NT])
            h_tile_matmuls(ht8[:rows8], rows8, n_groups * G + 3,
                           first=False, last=False)
            nc.scalar.activation(ht8[:rows8], ht8[:rows8], Sq,
                                 accum_out=hsq_sb[:rows8, NT - 1:NT])
            nc.scalar.dma_start(predo[r8:r8 + rows8, :], pd8[:rows8])

            # ---- main loop over groups: pred + loss1 + loss4 fused ----
            # stage-2 (masked square / H square) delayed one group so the
            # in-order DVE/ACT streams don't stall at intra-group barriers
            pending = None
            for g in range(n_groups):
                r0 = P * G * g
                fl = iop.tile([P, G, DIM3], f32, tag="flow")
                nc.sync.dma_start(
                    fl[:], flow[r0:r0 + P * G, :].rearrange(
                        "(a p) t -> p a t", p=P))
                mk = iop.tile([P, G, DIM3], u8, tag="mask")
                nc.sync.dma_start(
                    mk[:], maskt[r0:r0 + P * G, :].rearrange(
                        "(a p) t -> p a t", p=P))
                ht = iop.tile([P, G, HD], f32, tag="h")
                nc.gpsimd.dma_start(
                    ht[:], hslab[r0:r0 + P * G, :].rearrange(
                        "(a p) t -> p a t", p=P))
                pd = iop.tile([P, G, DIM3], f32, tag="predsb")

                for a in range(G):
                    i = G * g + a
                    pred_sub(fl[:, a, :], pd[:, a, :], r0 + P * a, P,
                             copy_on_dve=(a == 0))
                    h_tile_matmuls(ht[:, a, :], P, i,
                                   first=False,
                                   last=(g == n_groups - 1 and a == G - 1))
                nc.scalar.dma_start(
                    predo[r0:r0 + P * G, :].rearrange("(a p) t -> p a t", p=P),
                    pd[:])

                if pending is not None:
                    pfl, pmk, pht, pg = pending
                    nc.vector.copy_predicated(pfl[:], pmk[:], zeros[:])
                    nc.scalar.activation(pfl[:], pfl[:], Sq,
                                         accum_out=rs_sb[:, pg:pg + 1])
                    nc.scalar.activation(pht[:], pht[:], Sq,
                                         accum_out=hsq_sb[:, pg:pg + 1])
                pending = (fl, mk, ht, g)

            pfl, pmk, pht, pg = pending
            nc.vector.copy_predicated(pfl[:], pmk[:], zeros[:])
            nc.scalar.activation(pfl[:], pfl[:], Sq,
                                 accum_out=rs_sb[:, pg:pg + 1])
            nc.scalar.activation(pht[:], pht[:], Sq,
                                 accum_out=hsq_sb[:, pg:pg + 1])


            # ---- epilogue: small outputs ----
            nc.sync.dma_start(hsqo[:, :], hsq_sb[:])
            hd_sb = constp.tile([1, HD], f32)
            nc.scalar.copy(hd_sb[:], hd_ps[:])
            nc.sync.dma_start(hdo[:, :], hd_sb[:])
            nc.sync.dma_start(rssq[:, :], rs_sb[:])

    _split_multi_waits(nc, mybir)
    return nc


def _get_program():
    if "nc" not in _CACHE:
        _CACHE["nc"] = _build_program()
    return _CACHE["nc"]


def _sample(mu, rho, eps):
    mu = np.asarray(mu, np.float32)
    rho = np.asarray(rho, np.float32)
    eps = np.asarray(eps, np.float32)
    return mu + np.log1p(np.exp(rho)) * eps


def _prep_inputs(flow_missing, flow_missing_mask, heter_time_unmasked,
                 N, D, T, N_bias, D_bias, T_bias):
    """Build the per-core input maps."""
    flow = np.ascontiguousarray(np.asarray(flow_missing, np.float32))
    mask = np.asarray(flow_missing_mask)
    if mask.dtype != np.uint8:
        mask = mask.astype(np.uint8)
    ht = np.ascontiguousarray(np.asarray(heter_time_unmasked, np.float32))

    # augmented factors: pred = ND_aug @ T_aug^T
    ndkr = (N[:, None, :] * D[None, :, :]).reshape(DIM1 * DIM2, RANK)
    nbdb = (N_bias[:, None] + D_bias[None, :]).reshape(DIM1 * DIM2)
    nd_aug = np.empty((DIM1 * DIM2, KAUG), np.float32)
    nd_aug[:, :RANK] = ndkr
    nd_aug[:, RANK] = 1.0
    nd_aug[:, RANK + 1] = nbdb
    ndaugT = np.ascontiguousarray(nd_aug.T)          # [32, 40000]

    t_aug = np.empty((DIM3, KAUG), np.float32)
    t_aug[:, :RANK] = T
    t_aug[:, RANK] = T_bias
    t_aug[:, RANK + 1] = 1.0
    taugT = np.ascontiguousarray(t_aug.T)            # [32, 500]

    nnt = (N @ N.T).astype(np.float32)               # [200, 200]

    in_maps = []
    for c in range(NCORES):
        n0 = ROWS_N * c
        nn_flat = np.ascontiguousarray(nnt[n0:n0 + ROWS_N]).reshape(SLAB)
        nn_cols = np.zeros((P, NT), np.float32)
        nn_cols.T.reshape(-1)[:SLAB] = nn_flat       # col j = rows 128j..128j+128
        in_maps.append({
            "flow": flow[n0:n0 + ROWS_N].reshape(SLAB, DIM3),
            "maskt": np.ascontiguousarray(
                mask[n0:n0 + ROWS_N].reshape(SLAB, DIM3)),
            "hslab": ht[n0:n0 + ROWS_N].reshape(SLAB, HD),
            "ndaugT": np.ascontiguousarray(
                ndaugT[:, SLAB * c:SLAB * (c + 1)]),
            "taugT": taugT,
            "nntc": nn_cols,
        })
    return in_maps, nnt


def kernel(flow_missing, flow_missing_mask, heter_spatial_unmasked,
           heter_time_unmasked,
           N_mu, N_rho, eps_N, D_mu, D_rho, eps_D, T_mu, T_rho, eps_T,
           Nb_mu, Nb_rho, eps_Nb, Db_mu, Db_rho, eps_Db,
           Tb_mu, Tb_rho, eps_Tb,
           _trace=False, _trace_kwargs=None):
    from concourse.bass_utils import run_bass_kernel_spmd

    N = _sample(N_mu, N_rho, eps_N)
    D = _sample(D_mu, D_rho, eps_D)
    T = _sample(T_mu, T_rho, eps_T)
    N_bias = _sample(Nb_mu, Nb_rho, eps_Nb)
    D_bias = _sample(Db_mu, Db_rho, eps_Db)
    T_bias = _sample(Tb_mu, Tb_rho, eps_Tb)

    in_maps, nnt = _prep_inputs(flow_missing, flow_missing_mask,
                                heter_time_unmasked,
                                N, D, T, N_bias, D_bias, T_bias)

    nc = _get_program()
    res = run_bass_kernel_spmd(nc, in_maps, list(range(NCORES)),
                               trace=_trace, **(_trace_kwargs or {}))
    _CACHE["last_res"] = res
    results = res.results

    pred = np.concatenate(
        [results[c]["predo"].reshape(ROWS_N, DIM2, DIM3)
         for c in range(NCORES)], axis=0)

    # loss1
    ssq1 = np.float64(0.0)
    for c in range(NCORES):
        ssq1 += results[c]["rssq"].astype(np.float64).sum()
    loss1 = np.sqrt(ssq1)

    # loss2 (host, tiny)
    def fro(x):
        return np.sqrt(np.sum(np.asarray(x, np.float64) ** 2))
    loss2 = LAMDA * (fro(N) + fro(D) + fro(T))

    # loss3 (host, tiny)
    hs = np.asarray(heter_spatial_unmasked, np.float64)
    loss3 = LAMDA * fro(hs - nnt.astype(np.float64))

    # loss4 from device partials
    sumD = D.astype(np.float64).sum(axis=1)          # [200]
    ssq_ht = np.float64(0.0)
    hd_tot = np.zeros(HD, np.float64)
    for c in range(NCORES):
        ssq_ht += results[c]["hsqo"].astype(np.float64).sum()
        hd_tot += results[c]["hdo"].astype(np.float64).reshape(HD)
    ssq4 = (ssq_ht - 2.0 * np.dot(sumD, hd_tot)
            + np.sum(nnt.astype(np.float64) ** 2) * np.sum(sumD ** 2))
    loss4 = LAMDA * np.sqrt(ssq4)

    loss = np.float32(loss1 + loss2 + loss3 + loss4)
    return loss, pred


if __name__ == "__main__":
    # quick self-exercise with random data
    rng = np.random.default_rng(0)
    ins = {
        "flow_missing": rng.standard_normal((DIM1, DIM2, DIM3), np.float32),
        "flow_missing_mask": rng.integers(0, 2, (DIM1, DIM2, DIM3)).astype(bool),
        "heter_spatial_unmasked": rng.standard_normal((DIM1, DIM1), np.float32),
        "heter_time_unmasked": rng.standard_normal((DIM1, DIM1, DIM2), np.float32),
    }
    for nm, shp in (("N", (DIM1, RANK)), ("D", (DIM2, RANK)), ("T", (DIM3, RANK))):
        ins[f"{nm}_mu"] = (rng.standard_normal(shp) * 0.1).astype(np.float32)
        ins[f"{nm}_rho"] = (rng.standard_normal(shp) * 0.1 - 7).astype(np.float32)
        ins[f"eps_{nm}"] = rng.standard_normal(shp).astype(np.float32)
    for nm, shp in (("Nb", (DIM1,)), ("Db", (DIM2,)), ("Tb", (DIM3,))):
        ins[f"{nm}_mu"] = (rng.standard_normal(shp) * 0.1).astype(np.float32)
        ins[f"{nm}_rho"] = (rng.standard_normal(shp) * 0.1 - 7).astype(np.float32)
        ins[f"eps_{nm}"] = rng.standard_normal(shp).astype(np.float32)
    loss, pred = kernel(**ins)
    print("loss:", loss, "pred:", pred.shape, pred.dtype)


# revision 16
# speedup vs baseline: 1.2243x; 1.0118x over previous
"""Trainium2 Bass kernel for nn_HGTD (CP tensor-completion loss).

Computation (see reference):
  N,D,T,biases sampled via mu + softplus(rho)*eps  (tiny, done on host)
  pred = einsum('nr,dr,tr->ndt', N, D, T) + Nb+Db+Tb         [200,200,500]
  loss1 = ||where(~mask, pred-flow, 0)||_F
  loss2 = l*(||N||+||D||+||T||)
  loss3 = l*||hs - N@N.T||_F
  loss4 = l*||ht - (N@N.T)[:,:,None]*sumD||_F
  return loss1+loss2+loss3+loss4, pred

Device strategy (8 cores, shard dim0 of flow/mask/ht/pred -> 25 rows each):
  - biases folded into the CP matmul by augmenting rank 30 -> 32:
      ND_aug = [N (x) D | ones | Nb+Db],  T_aug = [T | Tb | ones]
    so PE produces pred tiles [128, 500] directly in PSUM.
  - per tile: ACT copies pred PSUM->SBUF (for DMA out), DVE computes
    diff = pred - flow, zeroes masked entries via copy_predicated, ACT
    squares with accum_out giving per-row partial sums of squares.
  - loss4 via quadratic expansion sum((ht - r1)^2) = sum(ht^2)
      - 2*sum_d sumD[d]*Hd[d] + sum(NNt^2)*sum(sumD^2), with
    sum(ht^2) and Hd = ht^T @ NNt_rows accumulated on PE in PSUM.
  - scalar finishing (traces, sqrt, tiny norms) on host in float64.
"""

import numpy as np

DIM1, DIM2, DIM3, RANK = 200, 200, 500, 30
LAMDA = 0.001
NCORES = 8
ROWS_N = DIM1 // NCORES          # 25 n-rows per core
SLAB = ROWS_N * DIM2             # 5000 nd-rows per core
P = 128
NT = (SLAB + P - 1) // P         # 40 tiles (39x128 + 1x8)
KAUG = RANK + 2                  # 32
HD = DIM2                        # heter_time last axis (200)
HCH = ((0, 128), (128, HD))      # d-chunks for loss4 PE pass

_CACHE = {}


def _split_multi_waits(nc, mybir):
    """walrus in this container accepts only ONE sem-wait per instruction.
    For any instruction Tile scheduled with >1 waits, hoist the extras onto
    freshly inserted NoOps on the same engine immediately before it (each
    engine executes its stream in block order, so all waits still complete
    before the instruction runs)."""
    n_split = 0
    for f in nc.m.functions:
        for bb in f.blocks:
            insts = list(bb.instructions)
            out = []
            changed = False
            for inst in insts:
                si = inst.sync_info
                if si is not None and si.on_wait and len(si.on_wait) > 1:
                    waits = list(si.on_wait)
                    for k, w in enumerate(waits[:-1]):
                        out.append(mybir.InstNoOp(
                            name=f"{inst.name}-sw{k}",
                            engine=inst.engine, ins=[], outs=[],
                            sync_info=mybir.SyncInfo(on_wait=[w],
                                                     on_update=[]),
                        ))
                    inst.sync_info = mybir.SyncInfo(
                        on_wait=[waits[-1]],
                        on_update=list(si.on_update or []),
                    )
                    changed = True
                    n_split += 1
                out.append(inst)
            if changed:
                bb.instructions = out
    return n_split


def _build_program():
    from concourse import bass, tile
    import concourse.mybir as mybir

    f32 = mybir.dt.float32
    f32r = mybir.dt.float32r
    u8 = mybir.dt.uint8
    Sub = mybir.AluOpType.subtract
    Sq = mybir.ActivationFunctionType.Square

    nc = bass.Bass("TRN2", target_bir_lowering=False, debug=False,
                   num_devices=NCORES)

    flow = nc.dram_tensor("flow", [SLAB, DIM3], f32, kind="ExternalInput")
    maskt = nc.dram_tensor("maskt", [SLAB, DIM3], u8, kind="ExternalInput")
    hslab = nc.dram_tensor("hslab", [SLAB, HD], f32, kind="ExternalInput")
    ndaugT = nc.dram_tensor("ndaugT", [KAUG, SLAB], f32r, kind="ExternalInput")
    taugT = nc.dram_tensor("taugT", [KAUG, DIM3], f32r, kind="ExternalInput")
    nntc = nc.dram_tensor("nntc", [P, NT], f32, kind="ExternalInput")

    predo = nc.dram_tensor("predo", [SLAB, DIM3], f32, kind="ExternalOutput")
    rssq = nc.dram_tensor("rssq", [P, NT], f32, kind="ExternalOutput")
    hsqo = nc.dram_tensor("hsqo", [P, NT], f32, kind="ExternalOutput")
    hdo = nc.dram_tensor("hdo", [1, HD], f32, kind="ExternalOutput")

    # group G row-tiles of 128 into one DMA (1MB flow transfers) to cut
    # Sync-sequencer DIRECT2D issue count; tail rows handled per-tile.
    G = 4
    n_groups = SLAB // (P * G)            # 9 full groups (4608 rows)
    tail0 = n_groups * P * G              # 4608
    tail_tiles = []
    r = tail0
    while r < SLAB:
        rows = min(P, SLAB - r)
        tail_tiles.append((r, rows))
        r += rows                          # (4608,128)(4736,128)(4864,128)(4992,8)

    with tile.TileContext(nc) as tc:
        with (
            tc.tile_pool(name="const", bufs=1) as constp,
            tc.tile_pool(name="io", bufs=5) as iop,
            tc.tile_pool(name="tail", bufs=1) as tailp,
            tc.tile_pool(name="ps", bufs=4, space="PSUM") as psp,
            tc.tile_pool(name="acc", bufs=1, space="PSUM") as accp,
        ):
            nd_sb = constp.tile([KAUG, SLAB], f32r)
            nc.sync.dma_start(nd_sb[:], ndaugT[:])
            ta_sb = constp.tile([KAUG, DIM3], f32r)
            nc.sync.dma_start(ta_sb[:], taugT[:])
            nn_sb = constp.tile([P, NT], f32)
            nc.sync.dma_start(nn_sb[:], nntc[:])
            zeros = constp.tile([P, G * DIM3], f32)
            nc.vector.memset(zeros[:], 0.0)
            rs_sb = constp.tile([P, NT], f32)
            nc.vector.memset(rs_sb[:], 0.0)

            hd_ps = accp.tile([1, HD], f32)
            hsq_sb = constp.tile([P, NT], f32)
            nc.vector.memset(hsq_sb[:], 0.0)

            def h_tile_matmuls(ht_ap, rows, j, first, last):
                # ht_ap: [rows, HD] slice; j = 128-row tile index for nnt col
                nc.tensor.matmul(hd_ps[:1, :], nn_sb[:rows, j:j + 1], ht_ap,
                                 start=first, stop=last,
                                 skip_group_check=True)

            def pred_sub(fl_ap, pd_ap, r0, rows, copy_on_dve):
                # matmul pred -> psum; copy psum->sbuf out; diff into fl
                pp = psp.tile([P, DIM3], f32, tag="predps")
                nc.tensor.matmul(pp[:rows], nd_sb[:, r0:r0 + rows], ta_sb[:],
                                 start=True, stop=True)
                if copy_on_dve:
                    nc.vector.tensor_copy(pd_ap, pp[:rows])
                else:
                    nc.scalar.copy(pd_ap, pp[:rows])
                nc.vector.tensor_tensor(fl_ap, pp[:rows], fl_ap, Sub)

            # ---- ragged tail first (hides under the startup ramp):
            # 3 full 128-row tiles as one grouped DMA + one 8-row tile ----
            t0r = n_groups * P * G                      # 4608
            fl3 = tailp.tile([P, 3, DIM3], f32, tag="tflow")
            nc.sync.dma_start(fl3[:], flow[t0r:t0r + 3 * P, :].rearrange(
                "(a p) t -> p a t", p=P))
            mk3 = tailp.tile([P, 3, DIM3], u8, tag="tmask")
            nc.sync.dma_start(mk3[:], maskt[t0r:t0r + 3 * P, :].rearrange(
                "(a p) t -> p a t", p=P))
            ht3 = tailp.tile([P, 3, HD], f32, tag="th")
            nc.gpsimd.dma_start(ht3[:], hslab[t0r:t0r + 3 * P, :].rearrange(
                "(a p) t -> p a t", p=P))
            pd3 = tailp.tile([P, 3, DIM3], f32, tag="tpred")
            for a in range(3):
                i = n_groups * G + a
                pred_sub(fl3[:, a, :], pd3[:, a, :], t0r + P * a, P,
                         copy_on_dve=(a == 0))
                h_tile_matmuls(ht3[:, a, :], P, i, first=(a == 0), last=False)
            nc.scalar.dma_start(
                predo[t0r:t0r + 3 * P, :].rearrange("(a p) t -> p a t", p=P),
                pd3[:])
            nc.vector.copy_predicated(fl3[:], mk3[:], zeros[:, :3 * DIM3])
            nc.scalar.activation(fl3[:], fl3[:], Sq,
                                 accum_out=rs_sb[:, NT - 2:NT - 1])
            nc.scalar.activation(ht3[:], ht3[:], Sq,
                                 accum_out=hsq_sb[:, NT - 2:NT - 1])

            r8, rows8 = tail_tiles[-1]                  # (4992, 8)
            fl8 = tailp.tile([P, DIM3], f32, tag="t8flow")
            nc.sync.dma_start(fl8[:rows8], flow[r8:r8 + rows8, :])
            mk8 = tailp.tile([P, DIM3], u8, tag="t8mask")
            nc.sync.dma_start(mk8[:rows8], maskt[r8:r8 + rows8, :])
            ht8 = tailp.tile([P, HD], f32, tag="t8h")
            nc.gpsimd.dma_start(ht8[:rows8], hslab[r8:r8 + rows8, :])
            pd8 = tailp.tile([P, DIM3], f32, tag="t8pred")
            pred_sub(fl8[:rows8], pd8[:rows8], r8, rows8, copy_on_dve=False)
            nc.vector.copy_predicated(fl8[:rows8], mk8[:rows8],
                                      zeros[:rows8, :DIM3])
            nc.scalar.activation(fl8[:rows8], fl8[:rows8], Sq,
                                 accum_out=rs_sb[:rows8, NT - 1:NT])
            h_tile_matmuls(ht8[:rows8], rows8, n_groups * G + 3,
                           first=False, last=False)
            nc.scalar.activation(ht8[:rows8], ht8[:rows8], Sq,
                                 accum_out=hsq_sb[:rows8, NT - 1:NT])
            nc.scalar.dma_start(predo[r8:r8 + rows8, :], pd8[:rows8])

            # ---- main loop over groups: pred + loss1 + loss4 fused ----
            # stage-2 (masked square / H square) delayed one group so the
            # in-order DVE/ACT streams don't stall at intra-group barriers
            pending = None
            for g in range(n_groups):
                r0 = P * G * g
                fl = iop.tile([P, G, DIM3], f32, tag="flow")
                nc.sync.dma_start(
                    fl[:], flow[r0:r0 + P * G, :].rearrange(
                        "(a p) t -> p a t", p=P))
                mk = iop.tile([P, G, DIM3], u8, tag="mask")
                nc.sync.dma_start(
                    mk[:], maskt[r0:r0 + P * G, :].rearrange(
                        "(a p) t -> p a t", p=P))
                ht = iop.tile([P, G, HD], f32, tag="h")
                nc.gpsimd.dma_start(
                    ht[:], hslab[r0:r0 + P * G, :].rearrange(
                        "(a p) t -> p a t", p=P))
                pd = iop.tile([P, G, DIM3], f32, tag="predsb")

                for a in range(G):
                    i = G * g + a
                    pred_sub(fl[:, a, :], pd[:, a, :], r0 + P * a, P,
                             copy_on_dve=(a == 0))
                    h_tile_matmuls(ht[:, a, :], P, i,
                                   first=False,
                                   last=(g == n_groups - 1 and a == G - 1))
                nc.scalar.dma_start(
                    predo[r0:r0 + P * G, :].rearrange("(a p) t -> p a t", p=P),
                    pd[:])

                if pending is not None:
                    pfl, pmk, pht, pg = pending
                    nc.vector.copy_predicated(pfl[:], pmk[:], zeros[:])
                    nc.scalar.activation(pfl[:], pfl[:], Sq,
                                         accum_out=rs_sb[:, pg:pg + 1])
                    nc.scalar.activation(pht[:], pht[:], Sq,
                                         accum_out=hsq_sb[:, pg:pg + 1])
                pending = (fl, mk, ht, g)

            pfl, pmk, pht, pg = pending
            nc.vector.copy_predicated(pfl[:], pmk[:], zeros[:])
            nc.scalar.activation(pfl[:], pfl[:], Sq,
                                 accum_out=rs_sb[:, pg:pg + 1])
            nc.scalar.activation(pht[:], pht[:], Sq,
                                 accum_out=hsq_sb[:, pg:pg + 1])


            # ---- epilogue: small outputs ----
            nc.sync.dma_start(hsqo[:, :], hsq_sb[:])
            hd_sb = constp.tile([1, HD], f32)
            nc.scalar.copy(hd_sb[:], hd_ps[:])
            nc.sync.dma_start(hdo[:, :], hd_sb[:])
            nc.sync.dma_start(rssq[:, :], rs_sb[:])

    _split_multi_waits(nc, mybir)
    return nc


def _get_program():
    if "nc" not in _CACHE:
        _CACHE["nc"] = _build_program()
    return _CACHE["nc"]


def _sample(mu, rho, eps):
    mu = np.asarray(mu, np.float32)
    rho = np.asarray(rho, np.float32)
    eps = np.asarray(eps, np.float32)
    return mu + np.log1p(np.exp(rho)) * eps


def _prep_inputs(flow_missing, flow_missing_mask, heter_time_unmasked,
                 N, D, T, N_bias, D_bias, T_bias):
    """Build the per-core input maps."""
    flow = np.ascontiguousarray(np.asarray(flow_missing, np.float32))
    mask = np.asarray(flow_missing_mask)
    if mask.dtype != np.uint8:
        mask = mask.astype(np.uint8)
    ht = np.ascontiguousarray(np.asarray(heter_time_unmasked, np.float32))

    # augmented factors: pred = ND_aug @ T_aug^T
    ndkr = (N[:, None, :] * D[None, :, :]).reshape(DIM1 * DIM2, RANK)
    nbdb = (N_bias[:, None] + D_bias[None, :]).reshape(DIM1 * DIM2)
    nd_aug = np.empty((DIM1 * DIM2, KAUG), np.float32)
    nd_aug[:, :RANK] = ndkr
    nd_aug[:, RANK] = 1.0
    nd_aug[:, RANK + 1] = nbdb
    ndaugT = np.ascontiguousarray(nd_aug.T)          # [32, 40000]

    t_aug = np.empty((DIM3, KAUG), np.float32)
    t_aug[:, :RANK] = T
    t_aug[:, RANK] = T_bias
    t_aug[:, RANK + 1] = 1.0
    taugT = np.ascontiguousarray(t_aug.T)            # [32, 500]

    nnt = (N @ N.T).astype(np.float32)               # [200, 200]

    in_maps = []
    for c in range(NCORES):
        n0 = ROWS_N * c
        nn_flat = np.ascontiguousarray(nnt[n0:n0 + ROWS_N]).reshape(SLAB)
        nn_cols = np.zeros((P, NT), np.float32)
        nn_cols.T.reshape(-1)[:SLAB] = nn_flat       # col j = rows 128j..128j+128
        in_maps.append({
            "flow": flow[n0:n0 + ROWS_N].reshape(SLAB, DIM3),
            "maskt": np.ascontiguousarray(
                mask[n0:n0 + ROWS_N].reshape(SLAB, DIM3)),
            "hslab": ht[n0:n0 + ROWS_N].reshape(SLAB, HD),
            "ndaugT": np.ascontiguousarray(
                ndaugT[:, SLAB * c:SLAB * (c + 1)]),
            "taugT": taugT,
            "nntc": nn_cols,
        })
    return in_maps, nnt


def kernel(flow_missing, flow_missing_mask, heter_spatial_unmasked,
           heter_time_unmasked,
           N_mu, N_rho, eps_N, D_mu, D_rho, eps_D, T_mu, T_rho, eps_T,
           Nb_mu, Nb_rho, eps_Nb, Db_mu, Db_rho, eps_Db,
           Tb_mu, Tb_rho, eps_Tb,
           _trace=False, _trace_kwargs=None):
    from concourse.bass_utils import run_bass_kernel_spmd

    N = _sample(N_mu, N_rho, eps_N)
    D = _sample(D_mu, D_rho, eps_D)
    T = _sample(T_mu, T_rho, eps_T)
    N_bias = _sample(Nb_mu, Nb_rho, eps_Nb)
    D_bias = _sample(Db_mu, Db_rho, eps_Db)
    T_bias = _sample(Tb_mu, Tb_rho, eps_Tb)

    in_maps, nnt = _prep_inputs(flow_missing, flow_missing_mask,
                                heter_time_unmasked,
                                N, D, T, N_bias, D_bias, T_bias)

    nc = _get_program()
    res = run_bass_kernel_spmd(nc, in_maps, list(range(NCORES)),
                               trace=_trace, **(_trace_kwargs or {}))
    _CACHE["last_res"] = res
    results = res.results

    pred = np.concatenate(
        [results[c]["predo"].reshape(ROWS_N, DIM2, DIM3)
         for c in range(NCORES)], axis=0)

    # loss1
    ssq1 = np.float64(0.0)
    for c in range(NCORES):
        ssq1 += results[c]["rssq"].astype(np.float64).sum()
    loss1 = np.sqrt(ssq1)

    # loss2 (host, tiny)
    def fro(x):
        return np.sqrt(np.sum(np.asarray(x, np.float64) ** 2))
    loss2 = LAMDA * (fro(N) + fro(D) + fro(T))

    # loss3 (host, tiny)
    hs = np.asarray(heter_spatial_unmasked, np.float64)
    loss3 = LAMDA * fro(hs - nnt.astype(np.float64))

    # loss4 from device partials
    sumD = D.astype(np.float64).sum(axis=1)          # [200]
    ssq_ht = np.float64(0.0)
    hd_tot = np.zeros(HD, np.float64)
    for c in range(NCORES):
        ssq_ht += results[c]["hsqo"].astype(np.float64).sum()
        hd_tot += results[c]["hdo"].astype(np.float64).reshape(HD)
    ssq4 = (ssq_ht - 2.0 * np.dot(sumD, hd_tot)
            + np.sum(nnt.astype(np.float64) ** 2) * np.sum(sumD ** 2))
    loss4 = LAMDA * np.sqrt(ssq4)

    loss = np.float32(loss1 + loss2 + loss3 + loss4)
    return loss, pred


if __name__ == "__main__":
    # quick self-exercise with random data
    rng = np.random.default_rng(0)
    ins = {
        "flow_missing": rng.standard_normal((DIM1, DIM2, DIM3), np.float32),
        "flow_missing_mask": rng.integers(0, 2, (DIM1, DIM2, DIM3)).astype(bool),
        "heter_spatial_unmasked": rng.standard_normal((DIM1, DIM1), np.float32),
        "heter_time_unmasked": rng.standard_normal((DIM1, DIM1, DIM2), np.float32),
    }
    for nm, shp in (("N", (DIM1, RANK)), ("D", (DIM2, RANK)), ("T", (DIM3, RANK))):
        ins[f"{nm}_mu"] = (rng.standard_normal(shp) * 0.1).astype(np.float32)
        ins[f"{nm}_rho"] = (rng.standard_normal(shp) * 0.1 - 7).astype(np.float32)
        ins[f"eps_{nm}"] = rng.standard_normal(shp).astype(np.float32)
    for nm, shp in (("Nb", (DIM1,)), ("Db", (DIM2,)), ("Tb", (DIM3,))):
        ins[f"{nm}_mu"] = (rng.standard_normal(shp) * 0.1).astype(np.float32)
        ins[f"{nm}_rho"] = (rng.standard_normal(shp) * 0.1 - 7).astype(np.float32)
        ins[f"eps_{nm}"] = rng.standard_normal(shp).astype(np.float32)
    loss, pred = kernel(**ins)
    print("loss:", loss, "pred:", pred.shape, pred.dtype)
